# revision 1
# baseline (speedup 1.0000x reference)
"""BitLinear (ternary-quantized linear) Trainium2 kernel.

Computes: out = x @ dequant(weight).T where dequant is per-group(128)
AbsMean ternary quantization (w_q in {-1,0,+1} times per-group scale).

Strategy (8 NeuronCores, column-parallel / tensor-parallel):
  - weight [O=11008, K=4096] is sharded by rows across 8 cores (1376 each).
  - x [B,S,K] -> [T=8192, K] is replicated to every core, pre-transposed on
    host to [K, T] so the contraction dim lands on SBUF partitions.
  - Each core dequantizes its weight shard on-chip (no division needed:
    round(w/s) clipped to [-1,1] == (w > s/2) - (w < -s/2)), applies the
    per-group scale (exact in fp16: products of {-1,0,1} with fp16(s)),
    transposes to [k, o] via the PE, and keeps the whole fp16 effective
    weight resident in SBUF, split into one piece per 512-wide out chunk.
  - x streams in t-tiles of 128 (f32 DMA + ACT-engine cast to fp16),
    accumulating in PSUM over the 32 k-groups.
  - To hide the dequant prefix, the first NW t-tiles process only chunk 0
    (ready after 4 of 11 o-tiles) while the rest of the dequant runs on
    the Vector/GpSimd engines; those t-tiles' remaining chunks run in a
    catch-up loop at the end.  Per-tile PE cost is identical either way.
  - Per-core output [T, 1376] (t-major); host concatenates along O.
"""

import os

import numpy as np

import concourse.bass as bass
import concourse.mybir as mybir
import concourse.tile as tile
from concourse import bacc
from concourse.bass_utils import run_bass_kernel_spmd
from concourse.masks import make_identity

P = 128
GROUP = 128
EPS = 1e-8

# Full problem shapes (hardcoded; harness calls kernel() with these).
FULL_B, FULL_S, FULL_K, FULL_O = 4, 2048, 4096, 11008
N_CORES = 8

LAST_RESULT = None  # BassKernelResults of the most recent run (for test.py)


def build_program(K, T, O_SHARD, mm_dt=mybir.dt.float16, n_warm=24):
    """One SPMD program, identical on every core (data differs per core).

    DRAM tensors:
      xt  [T, K] f32  ExternalInput -- x pre-packed on host (see pack_x) so
          that the per-t-tile load xt[tt*P+p, ko*G+t] = x[tt*P+t, ko*G+p]
          is one fully contiguous 2MB block (16KB per partition row)
      w   [O_SHARD, K] f32 ExternalInput (weight shard, natural layout)
      out [T, O_SHARD] f32 ExternalOutput
    """
    assert K % GROUP == 0 and T % P == 0
    KO = K // GROUP
    TB = min(8, KO)  # transpose-evac batch (ko per PSUM bank)
    assert KO % TB == 0 and KO % 2 == 0
    n_ttiles = T // P
    o_tiles = [(o0, min(P, O_SHARD - o0)) for o0 in range(0, O_SHARD, P)]
    OC = 512
    chunks = [(c0, min(OC, O_SHARD - c0)) for c0 in range(0, O_SHARD, OC)]
    NW = min(n_warm, n_ttiles) if len(chunks) > 1 else 0

    def chunk_of(o0):
        for ci, (c0, csz) in enumerate(chunks):
            if c0 <= o0 < c0 + csz:
                return ci
        raise AssertionError

    nc = bacc.Bacc("TRN2", target_bir_lowering=False, debug=False)
    xt = nc.dram_tensor("xt", [T, K], mybir.dt.float32, kind="ExternalInput").ap()
    w = nc.dram_tensor(
        "w", [O_SHARD, K], mybir.dt.float32, kind="ExternalInput"
    ).ap()
    out = nc.dram_tensor(
        "out", [T, O_SHARD], mybir.dt.float32, kind="ExternalOutput"
    ).ap()

    with tile.TileContext(nc) as tc:
        with (
            tc.tile_pool(name="wres", bufs=1) as wres,
            tc.tile_pool(name="const", bufs=1) as constp,
            tc.tile_pool(name="deq32", bufs=2) as deq32,
            tc.tile_pool(name="deq16", bufs=3) as deq16,
            tc.tile_pool(name="tiny", bufs=2) as tiny,
            tc.tile_pool(name="xf32", bufs=2) as xf32,
            tc.tile_pool(name="xin", bufs=2) as xin,
            tc.tile_pool(name="outp", bufs=2) as outp,
            tc.tile_pool(name="ps_tp", bufs=1, space="PSUM") as ps_tp,
            tc.tile_pool(name="ps_mm0", bufs=3, space="PSUM") as ps_mm0,
            tc.tile_pool(name="ps_mm", bufs=2, space="PSUM") as ps_mm,
        ):
            # Resident dequantized transposed weight, one piece per chunk.
            wbt = [wres.tile([P, KO, csz], mm_dt, tag=f"wbt{ci}", name=f"wbt{ci}")
                   for ci, (c0, csz) in enumerate(chunks)]
            ident = constp.tile([P, P], mm_dt)
            make_identity(nc, ident)

            # ---------------- dequant of one o-tile ----------------
            def dequant_tile(o0, osz):
                ci = chunk_of(o0)
                lo = o0 - chunks[ci][0]  # local col offset in wbt[ci]
                wt = deq32.tile([P, KO, GROUP], mybir.dt.float32, tag="wt")
                # w loads ride the ACT HWDGE ring so they never queue
                # behind the (much larger) x loads on the SP ring.
                nc.scalar.dma_start(
                    wt[:osz],
                    w[o0 : o0 + osz].rearrange("o (ko k) -> o ko k", k=GROUP),
                )
                # all-DVE dequant (keeps ACT free for the x casts): reduce
                # |w|, two broadcast compares, GpSimd subtract, scale-mult
                sums = tiny.tile([P, KO], mybir.dt.float32, tag="sums")
                nc.vector.tensor_reduce(
                    sums[:osz], wt[:osz],
                    axis=mybir.AxisListType.X, op=mybir.AluOpType.add,
                    apply_absolute_value=True,
                )
                # tpos = max(mean,EPS)/2; tneg = -tpos; s16 = fp16(max(mean,EPS))
                tpos = tiny.tile([P, KO], mybir.dt.float32, tag="tpos")
                nc.gpsimd.tensor_scalar(
                    tpos[:osz], sums[:osz], 0.5 / GROUP, 0.5 * EPS,
                    mybir.AluOpType.mult, mybir.AluOpType.max,
                )
                tneg = tiny.tile([P, KO], mybir.dt.float32, tag="tneg")
                nc.gpsimd.tensor_scalar_mul(tneg[:osz], tpos[:osz], -1.0)
                s16 = tiny.tile([P, KO], mm_dt, tag="s16")
                nc.gpsimd.tensor_scalar(
                    s16[:osz], sums[:osz], 1.0 / GROUP, EPS,
                    mybir.AluOpType.mult, mybir.AluOpType.max,
                )

                a = deq16.tile([P, KO, GROUP], mm_dt, tag="ab", name="a")
                b = deq16.tile([P, KO, GROUP], mm_dt, tag="ab", name="b")
                nc.vector.tensor_tensor(
                    a[:osz], wt[:osz],
                    tpos[:osz, :, None].to_broadcast((osz, KO, GROUP)),
                    mybir.AluOpType.is_gt,
                )
                nc.vector.tensor_tensor(
                    b[:osz], wt[:osz],
                    tneg[:osz, :, None].to_broadcast((osz, KO, GROUP)),
                    mybir.AluOpType.is_lt,
                )
                # q = a - b in {-1,0,1} on the otherwise idle GpSimd engine
                nc.gpsimd.tensor_tensor(
                    a[:osz], a[:osz], b[:osz], mybir.AluOpType.subtract
                )
                # wb = q * fp16(s)  (exact products of {-1,0,1} and fp16(s))
                nc.vector.tensor_tensor(
                    b[:osz], a[:osz],
                    s16[:osz, :, None].to_broadcast((osz, KO, GROUP)),
                    mybir.AluOpType.mult,
                )
                wb = b
                # transpose [o,k]->[k,o] through the PE, TB groups per bank
                for kb in range(0, KO, TB):
                    ps = ps_tp.tile([P, TB, P], mm_dt, tag="tp")
                    for j in range(TB):
                        nc.tensor.transpose(
                            ps[:, j, :osz], wb[:osz, kb + j, :],
                            ident[:osz, :osz],
                        )
                    nc.scalar.copy(
                        wbt[ci][:, kb : kb + TB, lo : lo + osz],
                        ps[:, :, :osz],
                    )

            # ---------------- one t-tile of matmuls for given chunks ----
            # host-packed: xt_r[tt, p, ko, t] = x[tt*P + t, ko*G + p];
            # per (tt, half) the source is contiguous (8KB per partition)
            xt_r = xt.rearrange("(tt p) (ko t) -> tt p ko t", p=P, t=P)
            KH = KO // 2

            def mm_ttile(tt, cis):
                # contiguous f32 halves per t-tile (8KB/partition rows thanks
                # to the host packing), cast to fp16 on ACT
                xb = xin.tile([P, KO, P], mm_dt, tag="xb")
                xf = xf32.tile([P, KO, P], mybir.dt.float32, tag="xf")
                nc.sync.dma_start(xf, xt_r[tt])
                nc.scalar.copy(xb, xf)
                t0 = tt * P
                w0 = chunks[cis[0]][0]
                wid = sum(chunks[ci][1] for ci in cis)
                ot_full = outp.tile([P, O_SHARD], mybir.dt.float32, tag="ot",
                                    name="ot")
                ot = ot_full[:, :wid]
                pss = {}
                for ci in cis:
                    pool = ps_mm0 if ci == 0 else ps_mm
                    ps = pool.tile([P, OC], mybir.dt.float32, tag=f"mm{ci}",
                                   name=f"mm{ci}")
                    pss[ci] = ps[:, : chunks[ci][1]]
                for ko in range(KO):
                    for ci in cis:
                        nc.tensor.matmul(
                            pss[ci],
                            lhsT=xb[:, ko, :],
                            rhs=wbt[ci][:, ko, :],
                            start=(ko == 0),
                            stop=(ko == KO - 1),
                        )
                for ci in cis:
                    c0, csz = chunks[ci]
                    nc.scalar.copy(ot[:, c0 - w0 : c0 - w0 + csz], pss[ci])
                nc.sync.dma_start(out[t0 : t0 + P, w0 : w0 + wid], ot)

            # ---------------- emission order ----------------
            all_cis = list(range(len(chunks)))
            rest_cis = all_cis[1:]
            for o0, osz in o_tiles:
                if chunk_of(o0) == 0:
                    dequant_tile(o0, osz)
            for o0, osz in o_tiles:
                if chunk_of(o0) != 0:
                    dequant_tile(o0, osz)
            for tt in range(NW):  # warmup: chunk 0 only
                mm_ttile(tt, [0])
            for tt in range(NW, n_ttiles):  # main: all chunks
                mm_ttile(tt, all_cis)
            for tt in range(NW):  # catch-up: remaining chunks
                mm_ttile(tt, rest_cis)

    nc.compile()
    return nc


def _run(nc, in_maps, trace=False):
    global LAST_RESULT
    res = run_bass_kernel_spmd(
        nc, in_maps, core_ids=list(range(len(in_maps))), trace=trace
    )
    LAST_RESULT = res
    return res


def pack_x(x2d):
    """[T, K] -> packed layout: H[tt*P+p, ko*G+t] = x2d[tt*P+t, ko*G+p]."""
    T, K = x2d.shape
    x4 = x2d.reshape(T // P, P, K // GROUP, GROUP)  # [tt, t, ko, p]
    return np.ascontiguousarray(x4.transpose(0, 3, 2, 1).reshape(T, K))


def kernel(x, weight):
    T = FULL_B * FULL_S
    K = FULL_K
    OS = FULL_O // N_CORES  # 1376
    x2d = pack_x(np.asarray(x, dtype=np.float32).reshape(T, K))
    w = np.asarray(weight, dtype=np.float32)

    nc = build_program(K, T, OS)
    in_maps = [
        {"xt": x2d, "w": np.ascontiguousarray(w[c * OS : (c + 1) * OS])}
        for c in range(N_CORES)
    ]
    trace = bool(os.environ.get("BASS_TRACE"))
    res = _run(nc, in_maps, trace=trace)
    full = np.concatenate(
        [res.results[c]["out"] for c in range(N_CORES)], axis=1
    )
    return np.ascontiguousarray(full.reshape(FULL_B, FULL_S, FULL_O))

